# revision 1
# baseline (speedup 1.0000x reference)
"""Trainium2 Bass kernel for nn_EquiformerLayer (Equiformer GNN message-passing layer).

Strategy
--------
Host (numpy, cheap):
  * Fold the weight chains: the leading irreps-Linear layers commute with the
    src-gather, and tp1 + lin_hidden fold into four 64x64 node-level maps
    (Wu, Wv, Wp, Wq).  Per edge only tp2 + lin_scalar matmuls remain.
  * Sort edges by dst, partition nodes into 8 contiguous ranges of 1250
    (one per core), and pad each 128-node window's edge list to whole
    128-edge tiles (uniform tile counts across cores so a single SPMD
    program serves all 8 cores).

Device (per core, no collectives):
  * Node stage: table[n] = [U|P|V0|Q0|V1|Q1|V2|Q2] (10112 x 512 bf16 in HBM)
    via PE matmuls on a host-pre-transposed atom-feature matrix.
  * Edge stage per 2048-edge supertile: dma_gather of table rows (bf16),
    spherical-harmonic scale factors, per-edge h0/h1/dot2 assembly on DVE,
    per-tile PE transposes + matmuls (data-stationary so outputs land in
    edge-major layout), softmax on ACT/DVE, and a one-hot matmul
    scatter-accumulate into a PSUM node-window.
  * Residual add + store of this core's 1250-node output range.
"""

import sys
import numpy as np

sys.path.insert(0, "/opt/trn_rl_repo")

import ml_dtypes  # noqa: E402
import concourse.bass as bass  # noqa: E402
import concourse.bacc as bacc  # noqa: E402
import concourse.mybir as mybir  # noqa: E402
import concourse.tile as tile  # noqa: E402
from concourse.bass_utils import run_bass_kernel_spmd  # noqa: E402
from concourse.library_config import mlp as mlp_lib  # noqa: E402

F32 = mybir.dt.float32
BF16 = mybir.dt.bfloat16
I16 = mybir.dt.int16
AL = mybir.AluOpType
AF = mybir.ActivationFunctionType

N_NODES = 10000
N_EDGES = 320000
N_CORES = 8
NPC = 1250            # nodes per core
WINDOWS = 10          # ceil(1250/128)
NPC_PAD = WINDOWS * 128   # 1280
TILE = 128
TPS = 16              # tiles per supertile
SQ3 = np.float32(np.sqrt(3.0))
INV_MUL = np.float32(1.0 / 8.0)
INV_TP = np.float32(1.0 / np.sqrt(128.0))
NCHUNK_NODES = (N_NODES + 127) // 128  # 79
NODES_PAD = NCHUNK_NODES * 128         # 10112
QUAD = 4              # tiles per psum quad in the edge stage


def _bf16(x):
    return np.asarray(x, np.float32).astype(ml_dtypes.bfloat16)


def host_prep(atom_feature, edge_vector, edge_index, w):
    """Returns (shared_inputs, per_core_inputs, meta)."""
    af = np.asarray(atom_feature, np.float32)
    ev = np.asarray(edge_vector, np.float32)
    ei = np.asarray(edge_index)
    src, dst = ei[0].astype(np.int64), ei[1].astype(np.int64)

    k = INV_MUL * INV_TP * INV_MUL
    Wu = w["lin_src_w0"] @ w["tp1_w00"] @ w["lin_hidden_w0"] * k
    Wv = w["lin_src_w1"] @ w["tp1_w11"] @ w["lin_hidden_w0"] * (k / SQ3)
    Wp = w["lin_src_w0"] @ w["tp1_w01"] @ w["lin_hidden_w1"] * k
    Wq = w["lin_src_w1"] @ w["tp1_w10"] @ w["lin_hidden_w1"] * k

    # node-stage moving operands: {U|P} from x0, {V|Q} from x1_m.
    # Stacked pairs so rhs base partitions match the lhsT slices (0 and 64).
    nW0 = _bf16(np.concatenate([Wu, Wp], axis=1))        # [64,128]
    nWm = _bf16(np.concatenate([Wv, Wq], axis=1))        # [64,128]

    # edge-stage matmul weights [K=128 stacked feats, N=128 outputs]
    w00 = w["tp2_w00"] * INV_TP
    w11 = w["tp2_w11"] * (INV_TP / SQ3)
    w01 = w["tp2_w01"] * INV_TP
    w10 = w["tp2_w10"] * INV_TP
    lsc = w["lin_scalar_w"] * INV_MUL
    z = np.zeros((64, 64), np.float32)
    wA = _bf16(np.block([[w00, w01], [w11, z]]))   # in [h0;dot2raw] -> {o0|t01b}
    wB = _bf16(np.block([[lsc, z], [z, w10]]))     # in [lrelu;h1_0] -> {sc|o1p0}
    wC = _bf16(np.block([[w10, z], [z, w10]]))     # in [h1_1;h1_2] -> {o1p1|o1p2}

    # pre-transposed atom features for the node stage: rows 0:64 x0^T,
    # 64+64m : 128+64m = x1_m^T ; cols padded to 10112
    x0 = af[:, :64]
    x1 = af[:, 64:].reshape(-1, 64, 3)
    afr = np.concatenate([x0, x1[:, :, 0], x1[:, :, 1], x1[:, :, 2]], axis=1)
    afT = np.zeros((256, NODES_PAD), np.float32)
    afT[:, :N_NODES] = afr.T
    afT = _bf16(afT)

    ident = _bf16(np.eye(128, dtype=np.float32))
    iota = _bf16(np.tile(np.arange(128, dtype=np.float32), (128, 1)))

    # ---- edge partition / sort / pad ----
    core_of = dst // NPC
    order = np.argsort(dst, kind="stable")

    per_core_edges = []
    for c in range(N_CORES):
        sel = order[core_of[order] == c]
        per_core_edges.append(sel)

    # per-core per-window tile counts -> uniform maxima
    win_tiles = np.zeros((N_CORES, WINDOWS), np.int64)
    win_edge_lists = [[None] * WINDOWS for _ in range(N_CORES)]
    for c in range(N_CORES):
        d = dst[per_core_edges[c]] - c * NPC
        wid = d // 128
        for wi in range(WINDOWS):
            e = per_core_edges[c][wid == wi]
            win_edge_lists[c][wi] = e
            win_tiles[c, wi] = (len(e) + TILE - 1) // TILE
    tw = win_tiles.max(axis=0)           # uniform per-window tile count
    T = int(tw.sum())
    T = ((T + TPS - 1) // TPS) * TPS     # pad to whole supertiles
    # extra pad tiles go to the last window
    tw_list = tw.tolist()
    tw_list[-1] += T - int(tw.sum())
    S = T // TPS

    tile_window = []
    for wi in range(WINDOWS):
        tile_window += [wi] * tw_list[wi]
    tile_window = np.asarray(tile_window)
    first_of_win = np.zeros(T, bool)
    last_of_win = np.zeros(T, bool)
    for wi in range(WINDOWS):
        idxs = np.flatnonzero(tile_window == wi)
        first_of_win[idxs[0]] = True
        last_of_win[idxs[-1]] = True

    per_core = []
    for c in range(N_CORES):
        src_pad = np.zeros(T * TILE, np.int16)
        dloc_pad = np.full(T * TILE, -1.0, np.float32)
        ev_pad = np.zeros((T * TILE, 3), np.float32)
        t0 = 0
        for wi in range(WINDOWS):
            e = win_edge_lists[c][wi]
            n = len(e)
            base = t0 * TILE
            src_pad[base:base + n] = src[e].astype(np.int16)
            dloc_pad[base:base + n] = (dst[e] - c * NPC - wi * 128).astype(np.float32)
            ev_pad[base:base + n] = ev[e]
            t0 += tw_list[wi]
        # device layouts
        idx_hbm = np.zeros((128, S * 128), np.int16)
        ev_hbm = np.zeros((128, S * 48), np.float32)
        dloc_hbm = np.zeros((128, S * 16), np.float32)
        for s in range(S):
            blk = slice(s * 2048, (s + 1) * 2048)
            for h in range(2):
                ids = src_pad[s * 2048 + h * 1024: s * 2048 + (h + 1) * 1024]
                idx_hbm[:, s * 128 + h * 64: s * 128 + (h + 1) * 64] = (
                    np.tile(ids.reshape(64, 16).T, (8, 1)))
            ev_hbm[:, s * 48:(s + 1) * 48] = (
                ev_pad[blk].reshape(TPS, 128, 3).transpose(1, 0, 2).reshape(128, 48))
            dloc_hbm[:, s * 16:(s + 1) * 16] = dloc_pad[blk].reshape(TPS, 128).T

        afrange = np.zeros((NPC_PAD, 256), np.float32)
        afrange[:NPC] = af[c * NPC:(c + 1) * NPC]

        per_core.append({
            "idx": idx_hbm,
            "ev": ev_hbm,
            "dloc": dloc_hbm,
            "afrange": afrange,
        })

    shared = {
        "afT": afT, "nW0": nW0, "nWm": nWm,
        "wA": wA, "wB": wB, "wC": wC,
        "ident": ident, "iota": iota,
    }
    meta = dict(S=S, T=T, tile_window=tile_window,
                first_of_win=first_of_win, last_of_win=last_of_win)
    return shared, per_core, meta


def build_program(meta, stage=6):
    S = meta["S"]
    T = meta["T"]
    tile_window = meta["tile_window"]
    first_of_win = meta["first_of_win"]
    last_of_win = meta["last_of_win"]

    nc = bacc.Bacc(None, target_bir_lowering=False)

    afT = nc.declare_dram_parameter("afT", [256, NODES_PAD], BF16, isOutput=False)
    nW0 = nc.declare_dram_parameter("nW0", [64, 128], BF16, isOutput=False)
    nWm = nc.declare_dram_parameter("nWm", [64, 128], BF16, isOutput=False)
    wA = nc.declare_dram_parameter("wA", [128, 128], BF16, isOutput=False)
    wB = nc.declare_dram_parameter("wB", [128, 128], BF16, isOutput=False)
    wC = nc.declare_dram_parameter("wC", [128, 128], BF16, isOutput=False)
    ident_d = nc.declare_dram_parameter("ident", [128, 128], BF16, isOutput=False)
    iota_d = nc.declare_dram_parameter("iota", [128, 128], BF16, isOutput=False)
    idx_d = nc.declare_dram_parameter("idx", [128, S * 128], I16, isOutput=False)
    ev_d = nc.declare_dram_parameter("ev", [128, S * 48], F32, isOutput=False)
    dloc_d = nc.declare_dram_parameter("dloc", [128, S * 16], F32, isOutput=False)
    afrange_d = nc.declare_dram_parameter("afrange", [NPC_PAD, 256], F32, isOutput=False)
    out_d = nc.declare_dram_parameter("out", [NPC_PAD, 256], F32, isOutput=True)

    table = nc.dram_tensor("table", [NODES_PAD, 512], BF16)

    nc.gpsimd.load_library(mlp_lib)

    with tile.TileContext(nc) as tc:
        with (
            tc.tile_pool(name="const", bufs=1) as cpool,
            tc.tile_pool(name="nodework", bufs=3) as npool,
            tc.tile_pool(name="gat", bufs=2) as gpool,
            tc.tile_pool(name="work", bufs=2) as wpool,
            tc.tile_pool(name="epsum", bufs=1, space="PSUM") as epsum,
            tc.tile_pool(name="wsum", bufs=2, space="PSUM") as wsum,
        ):
            # ---------------- constants / streams ----------------
            ident = cpool.tile([128, 128], BF16, tag="ident")
            iota = cpool.tile([128, 128], BF16, tag="iota")
            wa = cpool.tile([128, 128], BF16, tag="wa")
            wb = cpool.tile([128, 128], BF16, tag="wb")
            wc = cpool.tile([128, 128], BF16, tag="wc")
            nw0 = cpool.tile([64, 128], BF16, tag="nw0")
            nwm = cpool.tile([64, 128], BF16, tag="nwm")
            idx_sb = cpool.tile([128, S * 128], I16, tag="idx")
            ev_sb = cpool.tile([128, S * 48], F32, tag="ev")
            dloc_sb = cpool.tile([128, S * 16], F32, tag="dloc")
            agg = cpool.tile([128, WINDOWS, 256], F32, tag="agg")

            nc.sync.dma_start(out=ident[:], in_=ident_d[:])
            nc.sync.dma_start(out=iota[:], in_=iota_d[:])
            nc.sync.dma_start(out=wa[:], in_=wA[:])
            nc.sync.dma_start(out=wb[:], in_=wB[:])
            nc.sync.dma_start(out=wc[:], in_=wC[:])
            nWm_sb = nwm
            nc.sync.dma_start(out=nw0[:], in_=nW0[:])
            nc.sync.dma_start(out=nWm_sb[:], in_=nWm[:])
            nc.sync.dma_start(out=idx_sb[:], in_=idx_d[:])
            nc.sync.dma_start(out=ev_sb[:], in_=ev_d[:])
            nc.sync.dma_start(out=dloc_sb[:], in_=dloc_d[:])
            nc.sync.dma_start(
                out=agg[:],
                in_=afrange_d[:].rearrange("(w p) f -> p w f", p=128))

            # ---------------- node stage ----------------
            with tc.tile_pool(name="npsum", bufs=1, space="PSUM") as npsum:
              for cchunk in range(NCHUNK_NODES if stage >= 1 else 0):
                  cs = slice(cchunk * 128, (cchunk + 1) * 128)
                  xq = [npool.tile([64, 128], BF16, tag=f"xq{i}", name=f"xq{i}")
                        for i in range(4)]
                  for i in range(4):
                      nc.sync.dma_start(out=xq[i][:],
                                        in_=afT[64 * i:64 * (i + 1), cs])
                  ps = npsum.tile([128, 512], F32, tag="nps")
                  nc.tensor.matmul(out=ps[:, 0:128], lhsT=xq[0][:],
                                   rhs=nw0[:], start=True, stop=True)
                  for i in (1, 2, 3):
                      nc.tensor.matmul(out=ps[:, 128 * i:128 * (i + 1)],
                                       lhsT=xq[i][:], rhs=nWm_sb[:],
                                       start=True, stop=True)
                  tb = npool.tile([128, 512], BF16, tag="tb")
                  nc.vector.tensor_copy(out=tb[:], in_=ps[:])
                  nc.sync.dma_start(out=table[cs, :], in_=tb[:])

            # ---------------- edge stage ----------------
            psW = None
            for s in range(S if stage >= 2 else 0):
                g = gpool.tile([128, TPS, 512], BF16, tag="g")
                for h in range(2):
                    nc.gpsimd.dma_gather(
                        out_ap=g[:, h * 8:(h + 1) * 8, :], in_ap=table[:, :],
                        idxs_ap=idx_sb[:, s * 128 + h * 64:s * 128 + (h + 1) * 64],
                        num_idxs=1024, num_idxs_reg=1024,
                        elem_size=512)

                gU = g[:, :, 0:64]
                gP = g[:, :, 64:128]
                gV = [g[:, :, 128 + 128 * m:192 + 128 * m] for m in range(3)]
                gQ = [g[:, :, 192 + 128 * m:256 + 128 * m] for m in range(3)]

                if stage < 3:
                    continue
                # --- spherical harmonics ---
                evs = ev_sb[:, s * 48:(s + 1) * 48].rearrange("p (g m) -> p g m", m=3)
                sq = wpool.tile([128, TPS, 3], F32, tag="sq")
                nc.vector.tensor_tensor(out=sq[:], in0=evs, in1=evs, op=AL.mult)
                r2 = wpool.tile([128, TPS], F32, tag="r2")
                nc.vector.tensor_reduce(out=r2[:], in_=sq[:],
                                        axis=mybir.AxisListType.X, op=AL.add)
                rn = wpool.tile([128, TPS], F32, tag="rn")
                nc.scalar.sqrt(out=rn[:], in_=r2[:])
                nc.vector.tensor_scalar(out=rn[:], in0=rn[:], scalar1=1e-12,
                                        scalar2=None, op0=AL.add)
                nc.vector.reciprocal(out=rn[:], in_=rn[:])
                sh = wpool.tile([128, 3, TPS], F32, tag="sh")
                for m in range(3):
                    nc.vector.tensor_tensor(out=sh[:, m, :], in0=evs[:, :, m],
                                            in1=rn[:], op=AL.mult)
                shb = wpool.tile([128, 3, TPS, 64], BF16, tag="shb")
                for m in range(3):
                    nc.vector.tensor_scalar(
                        out=shb[:, m],
                        in0=sh[:, m, :].unsqueeze(-1).to_broadcast([128, TPS, 64]),
                        scalar1=float(SQ3), scalar2=None, op0=AL.mult)

                # --- h0 / h1 / dot2 assembly (edge-major, bf16) ---
                rA = wpool.tile([128, TPS, 128], BF16, tag="rA")
                rB = wpool.tile([128, TPS, 128], BF16, tag="rB")
                rC = wpool.tile([128, TPS, 128], BF16, tag="rC")
                tmp = wpool.tile([128, TPS, 64], BF16, tag="tmp")

                h0 = rA[:, :, 0:64]
                d2 = rA[:, :, 64:128]
                h1 = [rB[:, :, 64:128], rC[:, :, 0:64], rC[:, :, 64:128]]

                # h0 = gU + sum_m shb_m * gV_m
                nc.vector.tensor_tensor(out=h0, in0=shb[:, 0], in1=gV[0], op=AL.mult)
                nc.vector.tensor_tensor(out=h0, in0=h0, in1=gU, op=AL.add)
                for m in (1, 2):
                    nc.vector.tensor_tensor(out=tmp[:], in0=shb[:, m], in1=gV[m],
                                            op=AL.mult)
                    nc.vector.tensor_tensor(out=h0, in0=h0, in1=tmp[:], op=AL.add)
                # h1_m = shb_m * gP + gQ_m
                for m in range(3):
                    nc.vector.tensor_tensor(out=h1[m], in0=shb[:, m], in1=gP,
                                            op=AL.mult)
                    nc.vector.tensor_tensor(out=h1[m], in0=h1[m], in1=gQ[m],
                                            op=AL.add)
                # dot2_raw = sum_m shb_m * h1_m
                nc.vector.tensor_tensor(out=d2, in0=shb[:, 0], in1=h1[0], op=AL.mult)
                for m in (1, 2):
                    nc.vector.tensor_tensor(out=tmp[:], in0=shb[:, m], in1=h1[m],
                                            op=AL.mult)
                    nc.vector.tensor_tensor(out=d2, in0=d2, in1=tmp[:], op=AL.add)
                # lrelu(h0) = max(0.01*h0, h0)
                nc.vector.scalar_tensor_tensor(out=rB[:, :, 0:64], in0=h0,
                                               scalar=0.01, in1=h0,
                                               op0=AL.mult, op1=AL.max)

                if stage < 4:
                    continue
                # --- per-tile transposes + matmuls ---
                esb = wpool.tile([128, TPS, 3, 128], BF16, tag="esb")
                for q in range(TPS // QUAD):
                    psF = epsum.tile([128, QUAD, 3, 128], BF16, tag="psF",
                                     name="psF")
                    rsrc = [rA, rB, rC]
                    for j in range(QUAD):
                        t = q * QUAD + j
                        for i in range(3):
                            nc.tensor.transpose(out=psF[:, j, i, :],
                                                in_=rsrc[i][:, t, :],
                                                identity=ident[:])
                    rsb = npool.tile([128, QUAD, 3, 128], BF16, tag="rsb",
                                     name="rsb")
                    nc.vector.tensor_copy(out=rsb[:, :, 0, :], in_=psF[:, :, 0, :])
                    nc.scalar.activation(out=rsb[:, :, 1, :], in_=psF[:, :, 1, :],
                                         func=AF.Copy)
                    nc.vector.tensor_copy(out=rsb[:, :, 2, :], in_=psF[:, :, 2, :])

                    psE = epsum.tile([128, QUAD, 3, 128], F32, tag="psE",
                                     name="psE")
                    wmats = [wa, wb, wc]
                    for j in range(QUAD):
                        for i in range(3):
                            nc.tensor.matmul(out=psE[:, j, i, :],
                                             lhsT=rsb[:, j, i, :],
                                             rhs=wmats[i][:], start=True, stop=True)
                    qs = slice(q * QUAD, (q + 1) * QUAD)
                    nc.vector.tensor_copy(out=esb[:, qs, 0, :], in_=psE[:, :, 0, :])
                    nc.scalar.activation(out=esb[:, qs, 1, :], in_=psE[:, :, 1, :],
                                         func=AF.Copy)
                    nc.vector.tensor_copy(out=esb[:, qs, 2, :], in_=psE[:, :, 2, :])

                o0 = esb[:, :, 0, 0:64]
                t01b = esb[:, :, 0, 64:128]
                sc = esb[:, :, 1, 0:64]
                o1p = [esb[:, :, 1, 64:128], esb[:, :, 2, 0:64], esb[:, :, 2, 64:128]]

                if stage < 5:
                    continue
                # --- softmax over the 64 scalars (+192 zeros) ---
                e = wpool.tile([128, TPS, 64], BF16, tag="e")
                nc.scalar.activation(out=e[:], in_=sc, func=AF.Exp)
                zs = wpool.tile([128, TPS], F32, tag="zs")
                nc.vector.tensor_reduce(out=zs[:], in_=e[:],
                                        axis=mybir.AxisListType.X, op=AL.add)
                nc.vector.tensor_scalar(out=zs[:], in0=zs[:], scalar1=192.0,
                                        scalar2=None, op0=AL.add)
                nc.vector.reciprocal(out=zs[:], in_=zs[:])
                izb = wpool.tile([128, TPS, 64], BF16, tag="izb")
                nc.vector.tensor_scalar(
                    out=izb[:],
                    in0=zs[:].unsqueeze(-1).to_broadcast([128, TPS, 64]),
                    scalar1=1.0, scalar2=None, op0=AL.mult)

                # --- messages [128, TPS, 256] (vector part u-major, m inner) ---
                msgs = wpool.tile([128, TPS, 256], BF16, tag="msgs")
                act0 = wpool.tile([128, TPS, 64], BF16, tag="act0")
                nc.vector.tensor_tensor(out=act0[:], in0=e[:], in1=izb[:], op=AL.mult)
                nc.vector.tensor_tensor(out=msgs[:, :, 0:64], in0=act0[:], in1=o0,
                                        op=AL.mult)
                m1 = msgs[:, :, 64:256].rearrange("p g (u m) -> p g u m", m=3)
                o1t = wpool.tile([128, TPS, 64], BF16, tag="o1t")
                for m in range(3):
                    nc.vector.tensor_tensor(out=o1t[:], in0=shb[:, m], in1=t01b,
                                            op=AL.mult)
                    nc.vector.tensor_tensor(out=o1t[:], in0=o1t[:], in1=o1p[m],
                                            op=AL.add)
                    nc.vector.tensor_tensor(out=m1[:, :, :, m], in0=o1t[:],
                                            in1=izb[:], op=AL.mult)

                if stage < 6:
                    continue
                # --- scatter: one-hot matmul accumulate into node window ---
                # PSUM accumulation segments are local to this supertile: a
                # segment ends at a window change or the supertile end, then
                # is flushed (ACT copy out of PSUM, DVE add into agg).
                seg_start = True
                for t in range(TPS):
                    gidx = s * TPS + t
                    wi = int(tile_window[gidx])
                    oh = npool.tile([128, 128], BF16, tag="oh")
                    nc.vector.tensor_scalar(
                        out=oh[:], in0=iota[:],
                        scalar1=dloc_sb[:, s * 16 + t:s * 16 + t + 1],
                        scalar2=None, op0=AL.is_equal)
                    first_mm = seg_start
                    if seg_start:
                        psW = wsum.tile([128, 256], F32, tag="psW")
                        seg_start = False
                    seg_end = (t == TPS - 1) or (tile_window[gidx + 1] != wi)
                    nc.tensor.matmul(out=psW[:], lhsT=oh[:], rhs=msgs[:, t, :],
                                     start=first_mm, stop=seg_end,
                                     skip_group_check=True)
                    if seg_end:
                        fl = wpool.tile([128, 256], F32, tag="fl", name="fl")
                        nc.scalar.activation(out=fl[:], in_=psW[:], func=AF.Copy)
                        nc.vector.tensor_tensor(out=agg[:, wi, :],
                                                in0=agg[:, wi, :],
                                                in1=fl[:], op=AL.add)
                        seg_start = True

            # ---------------- output ----------------
            nc.sync.dma_start(
                out=out_d[:].rearrange("(w p) f -> p w f", p=128),
                in_=agg[:])

    nc.compile()
    return nc


def kernel(**inputs):
    wnames = ["lin_src_w0", "lin_src_w1", "lin_dst_w0", "lin_dst_w1",
              "tp1_w00", "tp1_w11", "tp1_w01", "tp1_w10",
              "tp2_w00", "tp2_w11", "tp2_w01", "tp2_w10",
              "lin_hidden_w0", "lin_hidden_w1", "lin_scalar_w"]
    w = {n: np.asarray(inputs[n], np.float32) for n in wnames}
    shared, per_core, meta = host_prep(
        inputs["atom_feature"], inputs["edge_vector"], inputs["edge_index"], w)

    nc = build_program(meta)
    in_maps = [{**shared, **pc} for pc in per_core]
    res = run_bass_kernel_spmd(nc, in_maps, list(range(N_CORES)))
    outs = [res.results[c]["out"][:NPC] for c in range(N_CORES)]
    out = np.concatenate(outs, axis=0).astype(np.float32)
    return out

